# revision 1
# baseline (speedup 1.0000x reference)
"""Trainium2 Bass kernel for nn_BiLSTM_CRF (CRF negative log-likelihood loss).

Problem: loss = mean_b( logZ_b - gold_b ) for a linear-chain CRF with
B=512 sequences, T=512 steps, K=128 tags (START=126, STOP=127).

Algorithm (per core, data-parallel over batch, 64 sequences/core):
  The log-semiring forward scan is computed in the exp domain so each step
  is one 128x128x64 TensorE matmul with a *fixed* stationary weight
  W = exp(transitions^T - c), where c is a constant per-step shift that
  keeps exp-domain magnitudes in fp32/bf16 range (the per-step log-growth
  of the partition function is ~c; measured drift stays within +-7 log
  units over all 512 steps, far inside bf16/fp32 exponent range):

      A_0 = onehot(START);  A_{t+1} = exp(feats_t) ⊙ (W @ A_t)
      logZ = log(colsum(A_T ⊙ exp(T[STOP,:] - c))) + (T+1)*c

  Gold-path score splits into:
    - emit  = sum_t feats[b,t,tags[b,t]]      -> on device (touches feats):
      one fused DVE scalar_tensor_tensor per 128-row block:
      (iota_k == tag_p) * feats_nat with accum_out giving the free-dim sum.
      Emit ops are interleaved 1:2 with scan steps so they fill the DVE
      gaps between the scan's PSUM-evacuation multiplies.
    - trans = sum_t T[tag_t,tag_{t-1}] (+STOP) -> on host (64KB table gather).

feats is shipped twice in bf16 (transposed [K, t-major(T,B)] for the scan's
matmul/exp pipeline, natural [B*T, K] for emit) -- 16MB/core of DMA, fully
hidden under the ~512-step scan chain.

The final mean over batch is a host-side sum of the 8 per-core partials.
"""

import numpy as np
import ml_dtypes

import concourse.bass as bass
from concourse import bacc
import concourse.mybir as mybir
import concourse.tile as tile
from concourse.tile import add_dep_helper
from concourse.alu_op_type import AluOpType

B, T, K = 512, 512, 128
NCORES = 8
BPC = B // NCORES  # 64 sequences per core
START, STOP = K - 2, K - 1

# Constant per-step shift: E[logZ]/T measured on the problem's data
# distribution (randn feats/transitions). Any value within ~0.1 of the true
# mean growth keeps the scan in range; measured drift with this value is
# [-6.7, +5.9] log units.
C_SHIFT = 5.826096

TSEG = 32               # scan timesteps per exp() segment
NSEG = T // TSEG
NBLK = BPC * T // 128   # 256 natural-layout 128-row blocks for emit score
BLK_GRP = 8             # natural blocks DMA'd together
F32 = mybir.dt.float32
BF16 = mybir.dt.bfloat16

_NC_CACHE = {}


def build_kernel():
    key = "nc"
    if key in _NC_CACHE:
        return _NC_CACHE[key]
    nc = bacc.Bacc(None, target_bir_lowering=False)
    AF = mybir.ActivationFunctionType

    featsT_d = nc.dram_tensor("featsT", [K, T * BPC], BF16, kind="ExternalInput")
    featsN_d = nc.dram_tensor("featsN", [BPC * T, K], BF16, kind="ExternalInput")
    tags_d = nc.dram_tensor("tagsT", [128, NBLK], BF16, kind="ExternalInput")
    trans_d = nc.dram_tensor("transT", [K, K], F32, kind="ExternalInput")
    out_d = nc.dram_tensor("out", [1, BPC], F32, kind="ExternalOutput")
    emit_d = nc.dram_tensor("emitcols", [128, NBLK], F32, kind="ExternalOutput")

    with tile.TileContext(nc) as tc:
        with (
            tc.tile_pool(name="const", bufs=1) as cpool,
            tc.tile_pool(name="big", bufs=1) as bigpool,
            tc.tile_pool(name="seg", bufs=2) as segpool,
            tc.tile_pool(name="nat", bufs=4) as natpool,
            tc.tile_pool(name="apool", bufs=3) as apool,
            tc.tile_pool(name="scr", bufs=8) as scrpool,
            tc.tile_pool(name="psum", bufs=3, space="PSUM") as psum_pool,
            tc.tile_pool(name="psumf", bufs=1, space="PSUM") as psum_fin,
        ):
            # ---- constants ----
            # transT input is transitions^T - c (host pre-shifted), so W and
            # stopcol are both exp() of it; logZ = logS + (T+1)*c on host.
            transT_s = cpool.tile([K, K], F32)
            nc.sync.dma_start(out=transT_s, in_=trans_d[:])
            W = cpool.tile([K, K], BF16)  # [prev, next] = exp(T^T - c)
            nc.scalar.activation(W, transT_s, AF.Exp)
            stopcol = cpool.tile([K, 1], F32)  # exp(T[STOP, k] - c) per partition k
            nc.scalar.activation(stopcol, transT_s[:, STOP : STOP + 1], AF.Exp)
            ones_b = cpool.tile([K, 1], BF16)
            nc.vector.memset(ones_b, 1.0)
            iota_k = cpool.tile([K, K], BF16)  # iota_k[p, j] = j
            nc.gpsimd.iota(
                iota_k,
                pattern=[[1, K]],
                base=0,
                channel_multiplier=0,
                allow_small_or_imprecise_dtypes=True,
            )
            emit_cols = bigpool.tile([128, NBLK], F32)

            # ---- resident transposed feats, t-major: col = t*BPC + b ----
            # Chunked plain DMAs so segment 0 is ready within a few us;
            # segment 0 itself lands in 4 sub-chunks so the scan can start
            # as soon as the first 8 timesteps are in.
            featsT = bigpool.tile([K, T * BPC], BF16)
            seg_cols = TSEG * BPC
            for q in range(4):
                sub = seg_cols // 4
                nc.sync.dma_start(
                    out=featsT[:, q * sub : (q + 1) * sub],
                    in_=featsT_d[:, q * sub : (q + 1) * sub],
                )
            tags_s = cpool.tile([128, NBLK], BF16)
            nc.sync.dma_start(out=tags_s, in_=tags_d[:])
            for s in range(1, NSEG):
                nc.sync.dma_start(
                    out=featsT[:, s * seg_cols : (s + 1) * seg_cols],
                    in_=featsT_d[:, s * seg_cols : (s + 1) * seg_cols],
                )

            # natural-layout feats blocks for the emit score (scalar engine
            # HWDGE queue so the sync queue stays on the scan-critical loads)
            nat_tiles = []
            for g in range(NBLK // BLK_GRP):
                nat = natpool.tile([128, BLK_GRP, K], BF16)
                nc.scalar.dma_start(
                    out=nat,
                    in_=featsN_d[
                        g * BLK_GRP * 128 : (g + 1) * BLK_GRP * 128, :
                    ].rearrange("(j p) k -> p j k", j=BLK_GRP),
                )
                nat_tiles.append(nat)

            # ---- A0 = onehot(START): fill 1.0 where partition == START ----
            # Two half-batch chains (32 seqs each) interleave so one chain's
            # DVE multiply overlaps the other's matmul latency.
            HB = BPC // 2
            A_half = []
            for h in range(2):
                Ah = apool.tile([K, HB], BF16, name=f"A0_{h}", tag=f"a0_{h}")
                nc.gpsimd.memset(Ah, 0.0)
                nc.gpsimd.affine_select(
                    out=Ah,
                    in_=Ah,
                    compare_op=AluOpType.not_equal,
                    fill=1.0,
                    base=-START,
                    channel_multiplier=1,
                    pattern=[[0, HB]],
                )
                A_half.append(Ah)

            # ---- the scan, with emit ops interleaved 1 per 2 steps ----
            # An explicit (non-sem) scheduler dep from each emit op onto the
            # preceding scan multiply keeps the DVE queue alternating
            # scan/emit; without it the scheduler front-loads all 256 emit
            # ops, stalling the scan chain ~90us.
            def emit_op(col, after_inst):
                g, j = divmod(col, BLK_GRP)
                scr = scrpool.tile([128, K], BF16, name="scr")
                ei = nc.vector.scalar_tensor_tensor(
                    out=scr,
                    in0=iota_k,
                    scalar=tags_s[:, col : col + 1],
                    in1=nat_tiles[g][:, j, :],
                    op0=AluOpType.is_equal,
                    op1=AluOpType.mult,
                    accum_out=emit_cols[:, col : col + 1],
                )
                if after_inst is not None:
                    add_dep_helper(
                        ei.ins, after_inst.ins, sync=False,
                        reason="spread emit over scan gaps",
                    )

            emit_idx = 0
            for s in range(NSEG):
                expF = segpool.tile([K, TSEG * BPC], F32)
                if s == 0:
                    for q in range(4):
                        sub = seg_cols // 4
                        nc.scalar.activation(
                            expF[:, q * sub : (q + 1) * sub],
                            featsT[:, q * sub : (q + 1) * sub],
                            AF.Exp,
                        )
                else:
                    nc.scalar.activation(
                        expF, featsT[:, s * seg_cols : (s + 1) * seg_cols], AF.Exp
                    )
                for ti in range(TSEG):
                    mi = None
                    for h in range(2):
                        psum_M = psum_pool.tile([K, HB], F32, name=f"pm{h}")
                        nc.tensor.matmul(
                            psum_M, W, A_half[h], start=True, stop=True
                        )
                        A_new = apool.tile(
                            [K, HB], BF16, name=f"A_new{h}", tag=f"a{h}"
                        )
                        mi = nc.vector.tensor_mul(
                            A_new,
                            psum_M,
                            expF[:, ti * BPC + h * HB : ti * BPC + (h + 1) * HB],
                        )
                        A_half[h] = A_new
                    t_global = s * TSEG + ti
                    if t_global % 2 == 1 and emit_idx < NBLK:
                        emit_op(emit_idx, mi)
                        emit_idx += 1
            while emit_idx < NBLK:
                emit_op(emit_idx, None)
                emit_idx += 1

            # ---- finalize: logS = log(colsum(A ⊙ stopcol)) ----
            Afin = apool.tile([K, BPC], BF16)
            for h in range(2):
                nc.vector.tensor_scalar_mul(
                    Afin[:, h * HB : (h + 1) * HB], A_half[h], stopcol
                )
            psum_S = psum_fin.tile([1, BPC], F32)
            nc.tensor.matmul(psum_S, ones_b, Afin, start=True, stop=True)
            logS = cpool.tile([1, BPC], F32)
            nc.scalar.activation(logS, psum_S, AF.Ln)
            nc.sync.dma_start(out=out_d[:], in_=logS)
            nc.sync.dma_start(out=emit_d[:], in_=emit_cols)

    nc.compile()
    nc.finalize()
    _NC_CACHE[key] = nc
    return nc


def prep_inputs(feats, tags, transitions):
    """Host-side marshalling: slice per core, cast bf16, build both layouts."""
    feats_bf = np.asarray(feats, dtype=np.float32).astype(ml_dtypes.bfloat16)
    tags64 = np.asarray(tags).astype(np.int64)
    transT = np.ascontiguousarray(
        np.asarray(transitions, dtype=np.float32).T - np.float32(C_SHIFT)
    )
    in_maps = []
    for c in range(NCORES):
        fc = feats_bf[c * BPC : (c + 1) * BPC]  # [BPC, T, K]
        fT = np.ascontiguousarray(fc.transpose(2, 1, 0).reshape(K, T * BPC))
        fN = np.ascontiguousarray(fc.reshape(BPC * T, K))
        tg = np.ascontiguousarray(
            tags64[c * BPC : (c + 1) * BPC]
            .reshape(NBLK, 128)
            .T.astype(ml_dtypes.bfloat16)
        )
        in_maps.append({"featsT": fT, "featsN": fN, "tagsT": tg, "transT": transT})
    return in_maps, tags64


def combine_outputs(results, tags64, transitions):
    """Host-side: per-core logS/emit partials + trans gold score -> loss."""
    Trf = np.asarray(transitions, dtype=np.float64)
    ext = np.concatenate([np.full((B, 1), START, np.int64), tags64], axis=1)
    trans_gold = Trf[ext[:, 1:], ext[:, :-1]].sum(axis=1) + Trf[STOP, ext[:, -1]]
    total = 0.0
    for c in range(NCORES):
        logS = results[c]["out"][0].astype(np.float64)  # [BPC]
        ecols = results[c]["emitcols"].astype(np.float64)  # [128, NBLK]
        emit_b = ecols.sum(axis=0).reshape(BPC, 4).sum(axis=1)
        logZ = logS + (T + 1) * C_SHIFT
        total += float(np.sum(logZ - emit_b - trans_gold[c * BPC : (c + 1) * BPC]))
    return np.asarray(total / B, dtype=np.float32)


def kernel(feats, tags, transitions):
    from concourse.bass_utils import run_bass_kernel_spmd

    nc = build_kernel()
    in_maps, tags64 = prep_inputs(feats, tags, transitions)
    res = run_bass_kernel_spmd(nc, in_maps, list(range(NCORES)))
    return combine_outputs(res.results, tags64, transitions)


if __name__ == "__main__":
    nc = build_kernel()
    print("kernel built and compiled OK")



# revision 4
# speedup vs baseline: 3.0953x; 3.0953x over previous
"""Trainium2 Bass kernel for nn_BiLSTM_CRF (CRF negative log-likelihood loss).

Problem: loss = mean_b( logZ_b - gold_b ) for a linear-chain CRF with
B=512 sequences, T=512 steps, K=128 tags (START=126, STOP=127).

Strategy: warmup time-split (no inter-core traffic).  The exp-domain scan
    A_{t+1} = exp(feats_t) * (W @ A_t),   W = exp(transitions^T - c)
is a product of positive matrices, which contracts directions fast (a
random-init vector converges to the true forward direction to ~1e-4 in
8 steps).  So core c runs the scan over global steps [64c-8, 64c+64) for
ALL 512 sequences, starting from all-ones (core 0 starts from the exact
onehot(START) at t=0, fed as input data).  Per-sequence log column sums
are read out at steps 8 / 64 / 72 (plus a stop-transition-weighted one at
72); the host telescopes segment growths into logZ:

    logZ = N64[core0] + sum_{c=1..6}(N72-N8)[c] + (N72stop-N8)[core7]
           + (T+1)*c_shift

Each core is 512 columns wide -> two 256-column chains fully hide the
matmul->multiply round trip; DVE (PSUM-evacuating multiply, ~833ns/step)
is the bottleneck engine, ScalarE exp()s feats in 8-step segments
underneath, and the 36MB of bf16 feats DMA hides under the scan.

Gold path score (emit + transition gathers) is computed on host, as the
baseline already did for the transition part.
"""

import numpy as np
import ml_dtypes

import concourse.bass as bass
from concourse import bacc
import concourse.mybir as mybir
import concourse.tile as tile

B, T, K = 512, 512, 128
NCORES = 8
START, STOP = K - 2, K - 1

# Constant per-step shift keeping the exp-domain scan in range (mean
# per-step log growth of the partition function on randn feats/trans).
C_SHIFT = 5.826096

WARM = 8                  # warmup steps (direction converges ~1e-4)
SEG = T // NCORES         # 64 real steps per core
NSTEP = SEG + WARM        # 72 scan steps per core
NCOLS = B                 # all 512 sequences on every core
HC = NCOLS // 2           # 256-column chain width
TSEG = 8                  # steps per exp() segment
NSEG = NSTEP // TSEG      # 9
F32 = mybir.dt.float32
BF16 = mybir.dt.bfloat16

_NC_CACHE = {}


def build_kernel():
    key = "nc"
    if key in _NC_CACHE:
        return _NC_CACHE[key]
    nc = bacc.Bacc(None, target_bir_lowering=False)
    AF = mybir.ActivationFunctionType

    featsT_d = nc.dram_tensor("featsT", [K, NSTEP * NCOLS], BF16, kind="ExternalInput")
    initA_d = nc.dram_tensor("initA", [K, NCOLS], BF16, kind="ExternalInput")
    trans_d = nc.dram_tensor("transT", [K, K], F32, kind="ExternalInput")
    norms_d = nc.dram_tensor("norms", [1, 4 * NCOLS], F32, kind="ExternalOutput")

    with tile.TileContext(nc) as tc:
        with (
            tc.tile_pool(name="const", bufs=1) as cpool,
            tc.tile_pool(name="big", bufs=1) as bigpool,
            tc.tile_pool(name="seg", bufs=2) as segpool,
            tc.tile_pool(name="apool", bufs=3) as apool,
            tc.tile_pool(name="psum", bufs=3, space="PSUM") as psum_pool,
            tc.tile_pool(name="psumn", bufs=2, space="PSUM") as psum_norm,
        ):
            # ---- constants ----
            # transT input is transitions^T - c (host pre-shifted)
            transT_s = cpool.tile([K, K], F32)
            nc.sync.dma_start(out=transT_s, in_=trans_d[:])
            W = cpool.tile([K, K], BF16)  # [prev, next] = exp(T^T - c)
            nc.scalar.activation(W, transT_s, AF.Exp)
            stopcol = cpool.tile([K, 1], BF16)  # exp(T[STOP, k] - c)
            nc.scalar.activation(stopcol, transT_s[:, STOP : STOP + 1], AF.Exp)
            ones_b = cpool.tile([K, 1], BF16)
            nc.vector.memset(ones_b, 1.0)
            norm_sb = cpool.tile([1, 4 * NCOLS], F32)

            # ---- resident transposed feats, t-major: col = t*NCOLS + b ----
            seg_cols = TSEG * NCOLS
            featsT = bigpool.tile([K, NSTEP * NCOLS], BF16)
            for q in range(2):  # first segment in halves for fast start
                sub = seg_cols // 2
                nc.sync.dma_start(
                    out=featsT[:, q * sub : (q + 1) * sub],
                    in_=featsT_d[:, q * sub : (q + 1) * sub],
                )
            A_half = []
            for h in range(2):
                Ah = apool.tile([K, HC], BF16, name=f"A0_{h}", tag=f"a{h}")
                nc.sync.dma_start(out=Ah, in_=initA_d[:, h * HC : (h + 1) * HC])
                A_half.append(Ah)
            for s in range(1, NSEG):
                nc.sync.dma_start(
                    out=featsT[:, s * seg_cols : (s + 1) * seg_cols],
                    in_=featsT_d[:, s * seg_cols : (s + 1) * seg_cols],
                )

            def colsum_norm(row, weights):
                """norm_sb[row] = ln(weights^T @ A) per column (sequence)."""
                psumN = psum_norm.tile([1, NCOLS], F32, name="pn", tag="pn")
                for h in range(2):
                    nc.tensor.matmul(
                        psumN[:, h * HC : (h + 1) * HC],
                        weights,
                        A_half[h],
                        start=True,
                        stop=True,
                    )
                nc.scalar.activation(norm_sb[:, row * NCOLS : (row + 1) * NCOLS], psumN, AF.Ln)

            # ---- the scan ----
            for s in range(NSEG):
                expF = segpool.tile([K, seg_cols], F32)
                if s == 0:
                    for q in range(2):
                        sub = seg_cols // 2
                        nc.scalar.activation(
                            expF[:, q * sub : (q + 1) * sub],
                            featsT[:, q * sub : (q + 1) * sub],
                            AF.Exp,
                        )
                else:
                    nc.scalar.activation(
                        expF, featsT[:, s * seg_cols : (s + 1) * seg_cols], AF.Exp
                    )
                for ti in range(TSEG):
                    t = s * TSEG + ti
                    for h in range(2):
                        psum_M = psum_pool.tile([K, HC], F32, name=f"pm{h}")
                        nc.tensor.matmul(psum_M, W, A_half[h], start=True, stop=True)
                        A_new = apool.tile([K, HC], BF16, name=f"A_new{h}", tag=f"a{h}")
                        nc.vector.tensor_mul(
                            A_new,
                            psum_M,
                            expF[:, ti * NCOLS + h * HC : ti * NCOLS + (h + 1) * HC],
                        )
                        A_half[h] = A_new
                    if t == WARM - 1:
                        colsum_norm(0, ones_b)   # N8
                    elif t == SEG - 1:
                        colsum_norm(1, ones_b)   # N64 (core 0's end)
                    elif t == NSTEP - 1:
                        colsum_norm(2, ones_b)   # N72
                        colsum_norm(3, stopcol)  # N72stop
            nc.sync.dma_start(out=norms_d[:], in_=norm_sb)

    nc.compile()
    nc.finalize()
    _NC_CACHE[key] = nc
    return nc


def prep_inputs(feats, tags, transitions):
    """Host-side marshalling: per-core time slices in [K, t-major] bf16."""
    feats_bf = np.asarray(feats, dtype=np.float32).astype(ml_dtypes.bfloat16)
    tags64 = np.asarray(tags).astype(np.int64)
    transT = np.ascontiguousarray(
        np.asarray(transitions, dtype=np.float32).T - np.float32(C_SHIFT)
    )
    # [K, T, B] once; per-core slices are views into it
    fTB = np.ascontiguousarray(feats_bf.transpose(2, 1, 0))
    ones_init = np.ones((K, NCOLS), dtype=ml_dtypes.bfloat16)
    onehot_init = np.zeros((K, NCOLS), dtype=ml_dtypes.bfloat16)
    onehot_init[START, :] = 1.0
    in_maps = []
    for c in range(NCORES):
        t0 = 0 if c == 0 else c * SEG - WARM
        fT = np.ascontiguousarray(fTB[:, t0 : t0 + NSTEP, :].reshape(K, NSTEP * NCOLS))
        init = onehot_init if c == 0 else ones_init
        in_maps.append({"featsT": fT, "initA": init, "transT": transT})
    return in_maps, tags64


def combine_outputs(results, tags64, feats, transitions):
    """Host-side: telescoped per-core growths + gold path score -> loss."""
    # logZ from per-core norm readouts
    logZ = np.zeros(B, dtype=np.float64)
    for c in range(NCORES):
        n = results[c]["norms"].astype(np.float64).reshape(4, B)  # N8 N64 N72 N72stop
        if c == 0:
            logZ += n[1]
        elif c == NCORES - 1:
            logZ += n[3] - n[0]
        else:
            logZ += n[2] - n[0]
    logZ += (T + 1) * C_SHIFT

    # gold path score entirely on host (cheap gathers)
    Trf = np.asarray(transitions, dtype=np.float64)
    ext = np.concatenate([np.full((B, 1), START, np.int64), tags64], axis=1)
    trans_gold = Trf[ext[:, 1:], ext[:, :-1]].sum(axis=1) + Trf[STOP, ext[:, -1]]
    fb = np.asarray(feats, dtype=np.float32).reshape(B * T, K)
    emit_gold = (
        fb[np.arange(B * T), tags64.reshape(-1)].astype(np.float64).reshape(B, T).sum(axis=1)
    )
    return np.asarray(np.mean(logZ - trans_gold - emit_gold), dtype=np.float32)


def kernel(feats, tags, transitions):
    from concourse.bass_utils import run_bass_kernel_spmd

    nc = build_kernel()
    in_maps, tags64 = prep_inputs(feats, tags, transitions)
    res = run_bass_kernel_spmd(nc, in_maps, list(range(NCORES)))
    return combine_outputs(res.results, tags64, feats, transitions)


if __name__ == "__main__":
    nc = build_kernel()
    print("kernel built and compiled OK")
